# revision 30
# baseline (speedup 1.0000x reference)
"""Trainium2 Bass kernel for nn_Encoder_23124103922122 (segment_reduce).

Math (per rank r of 6, labels lab_r[0..4095] in [0,256)):
    seg_r[b, g]  = sum_{i: lab_r[i]==g} F[b, i]          (segment sum)
    out[b, j, r] = seg_r[b, lab_r[j]]                     (gather back)
    out[b, j, 6] = F[b, j]                                (identity channel)

Implementation: both stages as one-hot matmuls on TensorE.
    stage 1: psum_seg[b, g] += F_T[i_tile].T @ M[i_tile, g]      (M one-hot of labels)
    stage 2: psum_out[b, j] = seg_T[g, b].T @ M_T[g, j]          (M_T one-hot, g on partitions)
One-hot matrices built on DVE via tensor_scalar(is_equal) from iota/label tables
that the host passes in as extra inputs.

Sharding: data-parallel over batch B=1024 -> 8 cores x 128 rows. Labels & tables
replicated. No cross-device communication.
"""

import sys

if "/opt/trn_rl_repo" not in sys.path:
    sys.path.insert(0, "/opt/trn_rl_repo")

from contextlib import ExitStack

import ml_dtypes
import numpy as np

import concourse.bass as bass
import concourse.mybir as mybir
import concourse.tile as tile
from concourse.bass import ts
from concourse.bass_utils import run_bass_kernel_spmd

B, N, R, G = 1024, 4096, 6, 256
NCORES = 8
BL = B // NCORES  # 128 batch rows per core
P = 128
NT = N // P  # 32 genus tiles
JC = 512  # stage-2 j-chunk width
NJ = N // JC
F32 = mybir.dt.float32
F32R = mybir.dt.float32r
BF16 = mybir.dt.bfloat16


def _r(ap):
    """View an fp32 AP as float32r for 4x-rate PE consumption."""
    return ap.bitcast(F32R) if ap.dtype == F32 else ap

# Compute dtype for the matmul operands (one-hots, F_T, seg_T).
# f32 = exact; bf16 = ~2x faster DVE/PE but ~3e-3 relative error.
COMPUTE_DT = F32
CD_NP = np.float32 if COMPUTE_DT == F32 else ml_dtypes.bfloat16

_cache: dict = {}

# Engine -> prefix of the semaphore names its compute instructions increment.
_ENGINE_SEM_PREFIX = {
    mybir.EngineType.PE: "PE",
    mybir.EngineType.DVE: "DVE",
    mybir.EngineType.Activation: "Activation",
    mybir.EngineType.Pool: "Pool",
    mybir.EngineType.SP: "SP",
}


def _legalize_waits(nc):
    """Walrus only accepts 1 sync-wait per instruction (2 on EventSemaphore),
    but the Tile scheduler can emit more. Post-pass:
      1. drop waits on the instruction's own engine semaphore that are already
         satisfied by same-engine program order (compute completion is in-order
         and sem targets are absolute), and
      2. hoist remaining excess waits onto EventSemaphore carrier instructions
         inserted just before the instruction on the same engine.
    """
    ev_id = 0
    for f in nc.m.functions:
        for blk in f.blocks:
            insts = blk.instructions
            sem_incs: dict = {}  # (engine, sem_name) -> cumulative inc in stream
            new_insts = []
            for inst in insts:
                si = inst.sync_info
                if si is not None and si.on_wait:
                    cap = 2 if isinstance(inst, mybir.InstEventSemaphore) else 1
                    eng = inst.engine
                    pfx = _ENGINE_SEM_PREFIX.get(eng)
                    kept = []
                    for w in si.on_wait:
                        sem_eng = w.ant_name.rsplit("_", 1)[0]
                        if (
                            pfx is not None
                            and sem_eng == pfx
                            and w.wait_mode == "sem-ge-imm"
                            and sem_incs.get((eng, w.ant_name), 0) >= w.wait_value
                        ):
                            continue  # satisfied by same-engine execution order
                        kept.append(w)
                    while len(kept) > cap:
                        ncarry = min(2, len(kept) - cap + 1)
                        carry, kept = kept[:ncarry], kept[ncarry:]
                        ev = mybir.InstEventSemaphore(
                            name=f"EVW-{ev_id}", ins=[], outs=[]
                        )
                        ev_id += 1
                        ev.engine = eng
                        ev.sync_info = mybir.SyncInfo(on_wait=carry, on_update=[])
                        new_insts.append(ev)
                    inst.sync_info = mybir.SyncInfo(
                        on_wait=kept, on_update=si.on_update
                    )
                si = inst.sync_info
                if si is not None:
                    for u in si.on_update:
                        if u.update_mode == "sem-inc":
                            key = (inst.engine, u.ant_name)
                            sem_incs[key] = sem_incs.get(key, 0) + u.update_value
                new_insts.append(inst)
            if len(new_insts) != len(insts):
                insts[:] = new_insts


def _build_nc():
    nc = bass.Bass("TRN2", debug=False, num_devices=NCORES)

    f_in = nc.dram_tensor("f_in", [BL, N], F32, kind="ExternalInput").ap()
    # tabs_f32[p, 0:2] = iota_p (p + 128*k); tabs_f32[p, 2 + r*NT + t] = labels[r, t*128+p]
    # (per-partition scalar operands for is_equal -- must be f32)
    tabs_f32 = nc.dram_tensor(
        "tabs_f32", [P, 2 + R * NT], F32, kind="ExternalInput"
    ).ap()
    # tabs_cd[p, 0:G] = iota_g (col index); tabs_cd[p, G:G+P] = identity
    tabs_cd = nc.dram_tensor(
        "tabs_cd", [P, G + P], COMPUTE_DT, kind="ExternalInput"
    ).ap()
    # lab_bf[r, j] = labels[r, j] (bf16, partition-broadcast source for stage 2)
    lab_bf = nc.dram_tensor("lab_bf", [R, N], BF16, kind="ExternalInput").ap()
    out = nc.dram_tensor("out", [BL, N, R + 1], F32, kind="ExternalOutput").ap()

    with ExitStack() as ctx:
        tc = ctx.enter_context(tile.TileContext(nc))

        const = ctx.enter_context(tc.tile_pool(name="const", bufs=1))
        fpool = ctx.enter_context(tc.tile_pool(name="fpool", bufs=1))
        mpool = ctx.enter_context(tc.tile_pool(name="mpool", bufs=24))
        segp = ctx.enter_context(tc.tile_pool(name="segp", bufs=1))
        mt2p = ctx.enter_context(tc.tile_pool(name="mt2p", bufs=8))
        outp = ctx.enter_context(tc.tile_pool(name="outp", bufs=3))
        ps_tr = ctx.enter_context(tc.tile_pool(name="ps_tr", bufs=2, space="PSUM"))

        # ---- constants + F load. Order matters: the tiny tables go first so
        # DVE mask-building starts ~1us in; then F (transposes); the big 6MB
        # lab_bc broadcast is gated behind the F load via a Pool-engine dep so
        # it streams during stage-1 compute instead of starving startup DMA. ----
        tf32_sb = const.tile([P, 2 + R * NT], F32)
        nc.sync.dma_start(tf32_sb[:], tabs_f32)
        tcd_sb = const.tile([P, G + P], COMPUTE_DT)
        nc.sync.dma_start(tcd_sb[:], tabs_cd)
        f_sb = fpool.tile([P, N], F32)
        for q in range(4):
            nc.sync.dma_start(
                f_sb[:, q * (N // 4) : (q + 1) * (N // 4)],
                f_in[:, q * (N // 4) : (q + 1) * (N // 4)],
            )
        # lab_bc[p, r, j] = labels[r, j] for every partition p.
        # The gate copy writes into lab_bc (WAW) so the 6MB broadcast DMA is
        # forced to start only after the F load has finished -- otherwise it
        # hogs the DMA engines while everything else waits on F/tables.
        lab_bc = const.tile([P, R, N], BF16)
        nc.gpsimd.tensor_copy(lab_bc[:, 0, 0:1], f_sb[:, N - 1 : N])
        nc.gpsimd.dma_start(lab_bc[:], lab_bf.partition_broadcast(P))
        iota_p_sb = tf32_sb[:, 0:2]
        labT_sb = tf32_sb[:, 2:]
        iota_g_sb = tcd_sb[:, 0:G]
        ident_sb = tcd_sb[:, G:]

        # Prewarm: absorb each const-DMA semaphore into the DVE/PE vector
        # clocks with one cheap op, so the TensorScalarPtr ops in the hot
        # loops never carry more than one sync wait (HW limit is 1 there).
        warm = const.tile([P, 4], COMPUTE_DT)
        nc.vector.tensor_copy(warm[:, 0:1], tf32_sb[:, 0:1])
        nc.vector.tensor_copy(warm[:, 1:2], tcd_sb[:, 0:1])
        with tc.tile_pool(name="ps_warm", bufs=1, space="PSUM") as ps_warm:
            wps = ps_warm.tile([P, P], COMPUTE_DT)
            nc.tensor.transpose(wps[:], ident_sb[:], ident_sb[:])
            nc.scalar.copy(warm[:, 3:4], wps[:, 0:1])
        f_cd = f_sb

        f_t = fpool.tile([P, N], F32R)  # col t*128.. holds transpose of tile t
        for t in range(NT):
            ps = ps_tr.tile([P, P], COMPUTE_DT, tag="tr")
            nc.tensor.transpose(ps[:], f_cd[:, ts(t, P)], ident_sb[:])
            nc.scalar.copy(f_t[:, ts(t, P)], ps[:])

        # ---- stage 1: seg[b, g] per rank, accumulated over the 32 genus tiles.
        # Rank-major so each rank's seg transposes overlap the next rank's
        # matmuls. Most one-hot masks are built on DVE (is_equal); every 6th
        # goes to the otherwise-idle ACT engine as relu(1 - |iota - lab|)
        # (exact for integer-valued inputs). ----
        seg_t = []
        m2_pre = {}
        with tc.tile_pool(name="ps_seg", bufs=1, space="PSUM") as ps_seg:
            seg_psum = [
                ps_seg.tile([P, G], F32, tag=f"seg{r}", name=f"seg_ps{r}")
                for r in range(R)
            ]
            for r in range(R):
                for t in range(NT):
                    col = r * NT + t
                    mt = mpool.tile([P, G], F32R, tag="m1")
                    if r >= 1 and t % 6 == 5:
                        tabs_ = mpool.tile([P, G], F32, tag="mabs")
                        nc.scalar.activation(
                            tabs_[:],
                            iota_g_sb[:],
                            mybir.ActivationFunctionType.Abs,
                            bias=labT_sb[:, col : col + 1],
                            scale=-1.0,
                        )
                        nc.scalar.activation(
                            mt[:],
                            tabs_[:],
                            mybir.ActivationFunctionType.Relu,
                            bias=1.0,
                            scale=-1.0,
                        )
                    else:
                        nc.vector.tensor_scalar(
                            mt[:],
                            iota_g_sb[:],
                            labT_sb[:, col : col + 1],
                            None,
                            op0=mybir.AluOpType.is_equal,
                        )
                    nc.tensor.matmul(
                        seg_psum[r][:],
                        f_t[:, ts(t, P)],
                        mt[:],
                        start=(t == 0),
                        stop=(t == NT - 1),
                    )

                # ---- seg -> seg_T (g on partitions) for this rank ----
                s_sb = segp.tile([P, G], COMPUTE_DT, tag=f"segsb{r}", name=f"ssb{r}")
                nc.scalar.copy(s_sb[:], seg_psum[r][:])
                st = segp.tile([P, G], F32R, tag=f"segT{r}", name=f"st{r}")
                for g in range(2):
                    ps = ps_tr.tile([P, P], COMPUTE_DT, tag="tr")
                    nc.tensor.transpose(ps[:], s_sb[:, ts(g, P)], ident_sb[:])
                    nc.scalar.copy(st[:, ts(g, P)], ps[:])
                seg_t.append(st)

        # ---- stage 2: out[b, j] = seg[b, lab[j]] per rank, interleave, store ----
        # absorb the lab_bc broadcast-DMA semaphore now (DVE was busy with
        # stage-1 masks while it streamed in)
        nc.vector.tensor_copy(warm[:, 2:3], lab_bc[:, 0, 0:1])
        # small chunks at the start (first out-DMA fires sooner) and at the
        # end (short final drain); big chunks in the middle for DMA efficiency
        widths = [512] * 7 + [256, 256]
        assert sum(widths) == N
        with tc.tile_pool(name="ps_o", bufs=4, space="PSUM") as ps_o:
            j0 = 0
            for c, w in enumerate(widths):
                o_sb = outp.tile([P, w, R + 1], F32, tag="osb", name=f"osb{c}")
                for r in range(R):
                    po = ps_o.tile([P, w], F32, tag="po", name=f"po{c}_{r}")
                    for g in range(2):
                        m2 = mt2p.tile([P, w], F32R, tag="m2", name=f"m2_{c}_{r}_{g}")
                        nc.vector.tensor_scalar(
                            m2[:],
                            lab_bc[:, r, j0 : j0 + w],
                            iota_p_sb[:, g : g + 1],
                            None,
                            op0=mybir.AluOpType.is_equal,
                        )
                        nc.tensor.matmul(
                            po[:],
                            seg_t[r][:, ts(g, P)],
                            m2[:],
                            start=(g == 0),
                            stop=(g == 1),
                        )
                    nc.scalar.copy(o_sb[:, :, r], po[:])
                nc.gpsimd.tensor_copy(o_sb[:, :, R], f_sb[:, j0 : j0 + w])
                nc.sync.dma_start(out[:, j0 : j0 + w, :], o_sb[:])
                j0 += w

    _legalize_waits(nc)
    return nc


def _host_tables():
    """tabs_cd aux table shared by all cores (cached)."""
    if "tabs_cd" not in _cache:
        iota_g = np.tile(np.arange(G, dtype=np.float64), (P, 1))
        ident = np.eye(P, dtype=np.float64)
        _cache["tabs_cd"] = np.concatenate([iota_g, ident], axis=1).astype(CD_NP)
    return _cache["tabs_cd"]


def kernel(F_genus: np.ndarray, labels: np.ndarray) -> np.ndarray:
    F_genus = np.ascontiguousarray(F_genus, dtype=np.float32)
    labels = np.ascontiguousarray(labels, dtype=np.int32)
    assert F_genus.shape == (B, N) and labels.shape == (R, N)

    tabs_cd = _host_tables()
    # labT[p, r*NT + t] = labels[r, t*128 + p]
    labT = np.transpose(labels.reshape(R, NT, P), (2, 0, 1)).reshape(P, R * NT)
    iota_p = np.arange(P, dtype=np.float64)[:, None] + 128.0 * np.arange(2)[None, :]
    tabs_f32 = np.concatenate([iota_p, labT], axis=1).astype(np.float32)
    lab_bf = labels.astype(ml_dtypes.bfloat16)

    in_maps = []
    for c in range(NCORES):
        in_maps.append(
            {
                "f_in": F_genus[c * BL : (c + 1) * BL],
                "tabs_f32": tabs_f32,
                "tabs_cd": tabs_cd,
                "lab_bf": lab_bf,
            }
        )

    # The first execution of a freshly compiled NEFF occasionally hits a
    # transient NRT_EXEC_UNIT_UNRECOVERABLE; a rebuild + retry recovers.
    last_err = None
    for attempt in range(3):
        try:
            if "nc" not in _cache:
                _cache["nc"] = _build_nc()
            res = run_bass_kernel_spmd(
                _cache["nc"], in_maps, core_ids=list(range(NCORES))
            )
            return np.concatenate([r["out"] for r in res.results], axis=0)
        except Exception as e:  # noqa: BLE001
            last_err = e
            _cache.pop("nc", None)
            import time as _time

            _time.sleep(3.0)
    raise last_err


# revision 33
# speedup vs baseline: 1.0005x; 1.0005x over previous
"""Trainium2 Bass kernel for nn_Encoder_23124103922122 (segment_reduce).

Math (per rank r of 6, labels lab_r[0..4095] in [0,256)):
    seg_r[b, g]  = sum_{i: lab_r[i]==g} F[b, i]          (segment sum)
    out[b, j, r] = seg_r[b, lab_r[j]]                     (gather back)
    out[b, j, 6] = F[b, j]                                (identity channel)

Implementation: both stages as one-hot matmuls on TensorE.
    stage 1: psum_seg[b, g] += F_T[i_tile].T @ M[i_tile, g]      (M one-hot of labels)
    stage 2: psum_out[b, j] = seg_T[g, b].T @ M_T[g, j]          (M_T one-hot, g on partitions)
One-hot matrices built on DVE via tensor_scalar(is_equal) from iota/label tables
that the host passes in as extra inputs (a few per rank on ACT via
relu(1-|iota-lab|), exact for integers). Matmul operands use float32r
(~13-bit-mantissa fp32) for 4x PE throughput vs plain fp32; transposes stay
plain fp32 (f32r transpose-mode is broken on HW). The [B, N, 7] channel
interleave is done by strided ACT copies PSUM->SBUF; output streams out in
j-chunks so the 14.6MB/core store overlaps stage-2 compute.

Sharding: data-parallel over batch B=1024 -> 8 cores x 128 rows. Labels & tables
replicated. No cross-device communication.

Cost-model timeline (per core): ~91us, roughly at the output-bandwidth
roofline (stage 1 is DVE/ACT mask-build bound ~40us; stage 2 is HBM-write
bound). Measured rel err vs fp32 reference: 1.5e-4.

Note: walrus in this container accepts at most ONE sync-wait per instruction
(two on EventSemaphore); _legalize_waits() post-processes the Tile-scheduled
program to satisfy that (drop provably-redundant same-engine waits, hoist the
rest onto EventSemaphore carriers).
"""

import sys

if "/opt/trn_rl_repo" not in sys.path:
    sys.path.insert(0, "/opt/trn_rl_repo")

from contextlib import ExitStack

import ml_dtypes
import numpy as np

import concourse.bass as bass
import concourse.mybir as mybir
import concourse.tile as tile
from concourse.bass import ts
from concourse.bass_utils import run_bass_kernel_spmd

B, N, R, G = 1024, 4096, 6, 256
NCORES = 8
BL = B // NCORES  # 128 batch rows per core
P = 128
NT = N // P  # 32 genus tiles
JC = 512  # stage-2 j-chunk width
NJ = N // JC
F32 = mybir.dt.float32
F32R = mybir.dt.float32r
BF16 = mybir.dt.bfloat16


def _r(ap):
    """View an fp32 AP as float32r for 4x-rate PE consumption."""
    return ap.bitcast(F32R) if ap.dtype == F32 else ap

# Compute dtype for the matmul operands (one-hots, F_T, seg_T).
# f32 = exact; bf16 = ~2x faster DVE/PE but ~3e-3 relative error.
COMPUTE_DT = F32
CD_NP = np.float32 if COMPUTE_DT == F32 else ml_dtypes.bfloat16

_cache: dict = {}

# Engine -> prefix of the semaphore names its compute instructions increment.
_ENGINE_SEM_PREFIX = {
    mybir.EngineType.PE: "PE",
    mybir.EngineType.DVE: "DVE",
    mybir.EngineType.Activation: "Activation",
    mybir.EngineType.Pool: "Pool",
    mybir.EngineType.SP: "SP",
}


def _legalize_waits(nc):
    """Walrus only accepts 1 sync-wait per instruction (2 on EventSemaphore),
    but the Tile scheduler can emit more. Post-pass:
      1. drop waits on the instruction's own engine semaphore that are already
         satisfied by same-engine program order (compute completion is in-order
         and sem targets are absolute), and
      2. hoist remaining excess waits onto EventSemaphore carrier instructions
         inserted just before the instruction on the same engine.
    """
    ev_id = 0
    for f in nc.m.functions:
        for blk in f.blocks:
            insts = blk.instructions
            sem_incs: dict = {}  # (engine, sem_name) -> cumulative inc in stream
            new_insts = []
            for inst in insts:
                si = inst.sync_info
                if si is not None and si.on_wait:
                    cap = 2 if isinstance(inst, mybir.InstEventSemaphore) else 1
                    eng = inst.engine
                    pfx = _ENGINE_SEM_PREFIX.get(eng)
                    kept = []
                    for w in si.on_wait:
                        sem_eng = w.ant_name.rsplit("_", 1)[0]
                        if (
                            pfx is not None
                            and sem_eng == pfx
                            and w.wait_mode == "sem-ge-imm"
                            and sem_incs.get((eng, w.ant_name), 0) >= w.wait_value
                        ):
                            continue  # satisfied by same-engine execution order
                        kept.append(w)
                    while len(kept) > cap:
                        ncarry = min(2, len(kept) - cap + 1)
                        carry, kept = kept[:ncarry], kept[ncarry:]
                        ev = mybir.InstEventSemaphore(
                            name=f"EVW-{ev_id}", ins=[], outs=[]
                        )
                        ev_id += 1
                        ev.engine = eng
                        ev.sync_info = mybir.SyncInfo(on_wait=carry, on_update=[])
                        new_insts.append(ev)
                    inst.sync_info = mybir.SyncInfo(
                        on_wait=kept, on_update=si.on_update
                    )
                si = inst.sync_info
                if si is not None:
                    for u in si.on_update:
                        if u.update_mode == "sem-inc":
                            key = (inst.engine, u.ant_name)
                            sem_incs[key] = sem_incs.get(key, 0) + u.update_value
                new_insts.append(inst)
            if len(new_insts) != len(insts):
                insts[:] = new_insts


def _build_nc():
    nc = bass.Bass("TRN2", debug=False, num_devices=NCORES)

    f_in = nc.dram_tensor("f_in", [BL, N], F32, kind="ExternalInput").ap()
    # tabs_f32[p, 0:2] = iota_p (p + 128*k); tabs_f32[p, 2 + r*NT + t] = labels[r, t*128+p]
    # (per-partition scalar operands for is_equal -- must be f32)
    tabs_f32 = nc.dram_tensor(
        "tabs_f32", [P, 2 + R * NT], F32, kind="ExternalInput"
    ).ap()
    # tabs_cd[p, 0:G] = iota_g (col index); tabs_cd[p, G:G+P] = identity
    tabs_cd = nc.dram_tensor(
        "tabs_cd", [P, G + P], COMPUTE_DT, kind="ExternalInput"
    ).ap()
    # lab_bf[r, j] = labels[r, j] (bf16, partition-broadcast source for stage 2)
    lab_bf = nc.dram_tensor("lab_bf", [R, N], BF16, kind="ExternalInput").ap()
    out = nc.dram_tensor("out", [BL, N, R + 1], F32, kind="ExternalOutput").ap()

    with ExitStack() as ctx:
        tc = ctx.enter_context(tile.TileContext(nc))

        const = ctx.enter_context(tc.tile_pool(name="const", bufs=1))
        fpool = ctx.enter_context(tc.tile_pool(name="fpool", bufs=1))
        mpool = ctx.enter_context(tc.tile_pool(name="mpool", bufs=24))
        segp = ctx.enter_context(tc.tile_pool(name="segp", bufs=1))
        mt2p = ctx.enter_context(tc.tile_pool(name="mt2p", bufs=8))
        outp = ctx.enter_context(tc.tile_pool(name="outp", bufs=3))
        ps_tr = ctx.enter_context(tc.tile_pool(name="ps_tr", bufs=2, space="PSUM"))

        # ---- constants + F load. Order matters: the tiny tables go first so
        # DVE mask-building starts ~1us in; then F (transposes); the big 6MB
        # lab_bc broadcast is gated behind the F load via a Pool-engine dep so
        # it streams during stage-1 compute instead of starving startup DMA. ----
        tf32_sb = const.tile([P, 2 + R * NT], F32)
        nc.sync.dma_start(tf32_sb[:], tabs_f32)
        tcd_sb = const.tile([P, G + P], COMPUTE_DT)
        nc.sync.dma_start(tcd_sb[:], tabs_cd)
        f_sb = fpool.tile([P, N], F32)
        f_dmas = [
            nc.sync.dma_start(
                f_sb[:, q * (N // 4) : (q + 1) * (N // 4)],
                f_in[:, q * (N // 4) : (q + 1) * (N // 4)],
            )
            for q in range(4)
        ]
        # lab_bc[p, r, j] = labels[r, j] for every partition p.
        # Explicit dep: the 6MB broadcast DMA must start only after the F load
        # has finished -- otherwise it hogs the DMA engines while everything
        # else waits on F/tables.
        lab_bc = const.tile([P, R, N], BF16)
        lab_dma = nc.gpsimd.dma_start(lab_bc[:], lab_bf.partition_broadcast(P))
        from concourse.tile import add_dep_helper

        add_dep_helper(
            lab_dma.ins, f_dmas[-1].ins, reason="delay lab_bc after F load"
        )
        iota_p_sb = tf32_sb[:, 0:2]
        labT_sb = tf32_sb[:, 2:]
        iota_g_sb = tcd_sb[:, 0:G]
        ident_sb = tcd_sb[:, G:]

        # Prewarm: absorb each const-DMA semaphore into the DVE/PE vector
        # clocks with one cheap op, so the TensorScalarPtr ops in the hot
        # loops never carry more than one sync wait (HW limit is 1 there).
        warm = const.tile([P, 4], COMPUTE_DT)
        nc.vector.tensor_copy(warm[:, 0:1], tf32_sb[:, 0:1])
        nc.vector.tensor_copy(warm[:, 1:2], tcd_sb[:, 0:1])
        with tc.tile_pool(name="ps_warm", bufs=1, space="PSUM") as ps_warm:
            wps = ps_warm.tile([P, P], COMPUTE_DT)
            nc.tensor.transpose(wps[:], ident_sb[:], ident_sb[:])
            nc.scalar.copy(warm[:, 3:4], wps[:, 0:1])
        f_cd = f_sb

        f_t = fpool.tile([P, N], F32R)  # col t*128.. holds transpose of tile t
        for t in range(NT):
            ps = ps_tr.tile([P, P], COMPUTE_DT, tag="tr")
            nc.tensor.transpose(ps[:], f_cd[:, ts(t, P)], ident_sb[:])
            nc.scalar.copy(f_t[:, ts(t, P)], ps[:])

        # ---- stage 1: seg[b, g] per rank, accumulated over the 32 genus tiles.
        # Rank-major so each rank's seg transposes overlap the next rank's
        # matmuls. Most one-hot masks are built on DVE (is_equal); every 6th
        # goes to the otherwise-idle ACT engine as relu(1 - |iota - lab|)
        # (exact for integer-valued inputs). ----
        seg_t = []
        m2_pre = {}
        with tc.tile_pool(name="ps_seg", bufs=1, space="PSUM") as ps_seg:
            seg_psum = [
                ps_seg.tile([P, G], F32, tag=f"seg{r}", name=f"seg_ps{r}")
                for r in range(R)
            ]
            for r in range(R):
                for t in range(NT):
                    col = r * NT + t
                    mt = mpool.tile([P, G], F32R, tag="m1")
                    if r >= 1 and t % 6 == 5:
                        tabs_ = mpool.tile([P, G], F32, tag="mabs")
                        nc.scalar.activation(
                            tabs_[:],
                            iota_g_sb[:],
                            mybir.ActivationFunctionType.Abs,
                            bias=labT_sb[:, col : col + 1],
                            scale=-1.0,
                        )
                        nc.scalar.activation(
                            mt[:],
                            tabs_[:],
                            mybir.ActivationFunctionType.Relu,
                            bias=1.0,
                            scale=-1.0,
                        )
                    else:
                        nc.vector.tensor_scalar(
                            mt[:],
                            iota_g_sb[:],
                            labT_sb[:, col : col + 1],
                            None,
                            op0=mybir.AluOpType.is_equal,
                        )
                    nc.tensor.matmul(
                        seg_psum[r][:],
                        f_t[:, ts(t, P)],
                        mt[:],
                        start=(t == 0),
                        stop=(t == NT - 1),
                    )

                # ---- seg -> seg_T (g on partitions) for this rank ----
                s_sb = segp.tile([P, G], COMPUTE_DT, tag=f"segsb{r}", name=f"ssb{r}")
                nc.scalar.copy(s_sb[:], seg_psum[r][:])
                st = segp.tile([P, G], F32R, tag=f"segT{r}", name=f"st{r}")
                for g in range(2):
                    ps = ps_tr.tile([P, P], COMPUTE_DT, tag="tr")
                    nc.tensor.transpose(ps[:], s_sb[:, ts(g, P)], ident_sb[:])
                    nc.scalar.copy(st[:, ts(g, P)], ps[:])
                seg_t.append(st)

        # ---- stage 2: out[b, j] = seg[b, lab[j]] per rank, interleave, store ----
        # absorb the lab_bc broadcast-DMA semaphore now (DVE was busy with
        # stage-1 masks while it streamed in)
        nc.vector.tensor_copy(warm[:, 2:3], lab_bc[:, 0, 0:1])
        # small chunks at the start (first out-DMA fires sooner) and at the
        # end (short final drain); big chunks in the middle for DMA efficiency
        widths = [512] * 7 + [256, 256]
        assert sum(widths) == N
        with tc.tile_pool(name="ps_o", bufs=4, space="PSUM") as ps_o:
            j0 = 0
            for c, w in enumerate(widths):
                o_sb = outp.tile([P, w, R + 1], F32, tag="osb", name=f"osb{c}")
                for r in range(R):
                    po = ps_o.tile([P, w], F32, tag="po", name=f"po{c}_{r}")
                    for g in range(2):
                        m2 = mt2p.tile([P, w], F32R, tag="m2", name=f"m2_{c}_{r}_{g}")
                        nc.vector.tensor_scalar(
                            m2[:],
                            lab_bc[:, r, j0 : j0 + w],
                            iota_p_sb[:, g : g + 1],
                            None,
                            op0=mybir.AluOpType.is_equal,
                        )
                        nc.tensor.matmul(
                            po[:],
                            seg_t[r][:, ts(g, P)],
                            m2[:],
                            start=(g == 0),
                            stop=(g == 1),
                        )
                    nc.scalar.copy(o_sb[:, :, r], po[:])
                nc.scalar.copy(o_sb[:, :, R], f_sb[:, j0 : j0 + w])
                nc.sync.dma_start(out[:, j0 : j0 + w, :], o_sb[:])
                j0 += w

    _legalize_waits(nc)
    return nc


def _host_tables():
    """tabs_cd aux table shared by all cores (cached)."""
    if "tabs_cd" not in _cache:
        iota_g = np.tile(np.arange(G, dtype=np.float64), (P, 1))
        ident = np.eye(P, dtype=np.float64)
        _cache["tabs_cd"] = np.concatenate([iota_g, ident], axis=1).astype(CD_NP)
    return _cache["tabs_cd"]


def kernel(F_genus: np.ndarray, labels: np.ndarray) -> np.ndarray:
    F_genus = np.ascontiguousarray(F_genus, dtype=np.float32)
    labels = np.ascontiguousarray(labels, dtype=np.int32)
    assert F_genus.shape == (B, N) and labels.shape == (R, N)

    tabs_cd = _host_tables()
    # labT[p, r*NT + t] = labels[r, t*128 + p]
    labT = np.transpose(labels.reshape(R, NT, P), (2, 0, 1)).reshape(P, R * NT)
    iota_p = np.arange(P, dtype=np.float64)[:, None] + 128.0 * np.arange(2)[None, :]
    tabs_f32 = np.concatenate([iota_p, labT], axis=1).astype(np.float32)
    lab_bf = labels.astype(ml_dtypes.bfloat16)

    in_maps = []
    for c in range(NCORES):
        in_maps.append(
            {
                "f_in": F_genus[c * BL : (c + 1) * BL],
                "tabs_f32": tabs_f32,
                "tabs_cd": tabs_cd,
                "lab_bf": lab_bf,
            }
        )

    # The first execution of a freshly compiled NEFF occasionally hits a
    # transient NRT_EXEC_UNIT_UNRECOVERABLE; a rebuild + retry recovers.
    last_err = None
    for attempt in range(3):
        try:
            if "nc" not in _cache:
                _cache["nc"] = _build_nc()
            res = run_bass_kernel_spmd(
                _cache["nc"], in_maps, core_ids=list(range(NCORES))
            )
            return np.concatenate([r["out"] for r in res.results], axis=0)
        except Exception as e:  # noqa: BLE001
            last_err = e
            _cache.pop("nc", None)
            import time as _time

            _time.sleep(3.0)
    raise last_err
